# revision 17
# baseline (speedup 1.0000x reference)
"""GPRGNN kernel for 8 Trainium2 NeuronCores (Bass/Tile).

Algorithm notes:
  reference: h0 = MLP(x); hidden = sum_k temp[k] * (D^-1/2 A D^-1/2)^k h0
  We propagate in g-space: g = D^-1/2 h. Then
     g_{k+1} = D^-1 * (A @ g_k)        (A = adjacency + self loops, unit weights)
     hidden  = D^1/2 * sum_k temp[k] g_k
  so per-edge norm weights vanish; each hop is a pure gather + segment-sum.

Sharding: nodes are permuted so core c owns 12544 destination slots
(12500 real nodes padded to 98 groups of 128). Nodes are assigned
round-robin by degree rank, and sorted by degree within a core, so the
128 dst nodes of a group have nearly identical in-degree -> the per-group
edge matrix [128, S_g] has ~no padding.

v2: gathers are batched: several groups (padded to a common slot count,
up to 64 slots/chunk) share one indirect DMA, amortizing the ~1us SWDGE
fixed cost (577us/hop vs 697us/hop measured). The segment-sum fold runs
on 3D APs across all groups of a chunk. Propagation state stays f32:
the HW indirect-DMA descriptor path only honors per-slot indices for
256B rows (f32 x 64ch); bf16 rows (128B) and dtype-cast variants return
corrupted gathers (verified empirically). MLP runs in bf16 (inputs and
weights) with f32 accumulation; phase C (log_softmax) is batched across
groups with single big reduce/exp instructions.
"""

import os
import sys

for _p in ("/opt/trn_rl_repo", "/opt/pypackages"):
    if _p not in sys.path:
        sys.path.insert(0, _p)

import numpy as np

N = 100_000
E = 3_200_000
F_IN = 512
H = 256
C = 64
K = int(os.environ.get("KV2_K", "10"))   # override only for timing runs
NCORES = 8
P = 128
G = 98                  # groups of 128 dst nodes per core
PC = G * P              # 12544 owned slots per core
NPAD = NCORES * PC      # 100352
CHUNK_SLOTS = int(os.environ.get("KV2_CHUNK", "64"))
QUAD = 4                # node-groups per MLP matmul batch (N=512 cols)
F_STT = os.environ.get("KV2_STT", "1") == "1"      # mixed-dtype scalar_tensor_tensor
F_FOLD3D = os.environ.get("KV2_FOLD3D", "1") == "1"  # batched 3D-AP fold
F_PHC = os.environ.get("KV2_PHC", "1") == "1"      # batched phase C
F_TBF = os.environ.get("KV2_TBF", "1") == "1"      # bf16 PE transpose / psum
F_GBF16 = os.environ.get("KV2_GBF16", "0") == "1"  # bf16 propagation state
F_GCAST = os.environ.get("KV2_GCAST", "0") == "1"  # f32 gather rows cast to bf16 dest
F_NOSELF = os.environ.get("KV2_NOSELF", "0") == "1"  # self-loops via DVE add, not gather

_profile_info = {}      # filled when KERNEL_TRACE=1 (for test.py)


def _host_prep(x, edge_index):
    """Permute nodes, partition+pad edges, build per-core arrays."""
    src = np.asarray(edge_index[0], dtype=np.int64)
    dst = np.asarray(edge_index[1], dtype=np.int64)

    deg = np.bincount(dst, minlength=N).astype(np.int64) + 1  # incl self loop
    order = np.argsort(deg, kind="stable")          # ascending degree
    ranks = np.arange(N, dtype=np.int64)
    core_of = ranks % NCORES
    local_of = ranks // NCORES
    new_id = np.empty(N, dtype=np.int64)
    new_id[order] = core_of * PC + local_of         # old id -> padded new id

    ns = new_id[src]
    nd = new_id[dst]
    if F_NOSELF:
        all_src, all_dst = ns, nd                   # self loop added on-chip
    else:
        all_src = np.concatenate([ns, new_id])      # + self loops
        all_dst = np.concatenate([nd, new_id])
    o = np.argsort(all_dst, kind="stable")
    s_sorted = np.ascontiguousarray(all_src[o])

    deg_new = np.bincount(all_dst, minlength=NPAD).astype(np.int64)
    S_g = deg_new.reshape(NCORES, G, P).max(axis=(0, 2)).astype(np.int64)  # [G]

    # chunks: consecutive groups padded to the chunk max S, bounded by
    # CHUNK_SLOTS total slots per chunk
    chunks = []          # (g_start, n_groups, S_bar, col_off)
    g0 = 0
    col = 0
    while g0 < G:
        smax = int(S_g[g0])
        nb = 1
        while (g0 + nb < G):
            s2 = max(smax, int(S_g[g0 + nb]))
            if (nb + 1) * s2 > CHUNK_SLOTS:
                break
            smax = s2
            nb += 1
        chunks.append((g0, nb, smax, col))
        col += nb * smax
        g0 += nb
    sum_s = col

    S_max = int(S_g.max())
    cum = np.concatenate([[0], np.cumsum(deg_new)]).astype(np.int64)
    # big[nid, j] = j-th src of node nid (pad -> owner's dummy slot, deg 0)
    zrow = (np.arange(NPAD, dtype=np.int64) // PC) * PC + (PC - 1)
    jj = np.arange(S_max, dtype=np.int64)[None, :]
    pos = np.minimum(cum[:-1][:, None] + jj, len(s_sorted) - 1)
    valid = jj < deg_new[:, None]
    big = np.where(valid, s_sorted[pos], zrow[:, None]).astype(np.int32)

    idx_blobs, xts, dinv_cols, dinv2_cols, sqd_cols = [], [], [], [], []
    real = np.zeros(NPAD, dtype=bool)
    real[new_id] = True
    deg_norm = deg_new + (1 if F_NOSELF else 0) * real.astype(np.int64)
    deg_f = deg_norm.astype(np.float64)
    with np.errstate(divide="ignore"):
        dinv_all = np.where(deg_norm > 0, 1.0 / np.sqrt(np.maximum(deg_f, 1e-12)), 0.0)
        dinv2_all = np.where(deg_norm > 0, 1.0 / np.maximum(deg_f, 1e-12), 0.0)
        sqd_all = np.where(deg_norm > 0, np.sqrt(deg_f), 0.0)

    from ml_dtypes import bfloat16

    for c in range(NCORES):
        rows = slice(c * PC, (c + 1) * PC)
        bc = big[rows]                               # [PC, S_max]
        dummy = c * PC + (PC - 1)
        blob = np.full((P, sum_s), dummy, dtype=np.int32)
        for (gs, nb, sbar, col0) in chunks:
            for i in range(nb):
                g = gs + i
                blk = bc[g * P:(g + 1) * P, : S_g[g]]    # [128, S_g]
                c0 = col0 + i * sbar
                blob[:, c0:c0 + int(S_g[g])] = blk
        idx_blobs.append(blob)

        own_old = order[ranks[core_of == c]]         # old ids, local order asc
        xt = np.zeros((F_IN, PC), dtype=np.float32)
        xt[:, : len(own_old)] = np.asarray(x)[own_old].T
        xts.append(np.ascontiguousarray(xt.astype(bfloat16)))

        dinv_cols.append(np.ascontiguousarray(
            dinv_all[rows].reshape(G, P).T.astype(np.float32)))   # [128, G]
        dinv2_cols.append(np.ascontiguousarray(
            dinv2_all[rows].reshape(G, P).T.astype(np.float32)))
        sqd_cols.append(np.ascontiguousarray(
            sqd_all[rows].reshape(G, P).T.astype(np.float32)))

    return (new_id, chunks, sum_s, idx_blobs, xts,
            dinv_cols, dinv2_cols, sqd_cols)


def _build_program(chunks, sum_s, temps):
    import concourse.bass as bass
    import concourse.bacc as bacc
    import concourse.mybir as mybir
    import concourse.tile as tile
    from concourse.masks import make_identity

    f32 = mybir.dt.float32
    bf16 = mybir.dt.bfloat16
    i32 = mybir.dt.int32
    AF = mybir.ActivationFunctionType
    ALU = mybir.AluOpType

    nc = bacc.Bacc(None, num_devices=NCORES)

    xt_d = nc.dram_tensor("xt", [F_IN, PC], bf16, kind="ExternalInput")
    w1t_d = nc.dram_tensor("w1t", [F_IN, H], bf16, kind="ExternalInput")
    b1_d = nc.dram_tensor("b1", [H], f32, kind="ExternalInput")
    w2t_d = nc.dram_tensor("w2t", [H, C], bf16, kind="ExternalInput")
    b2_d = nc.dram_tensor("b2", [C], f32, kind="ExternalInput")
    dinv_d = nc.dram_tensor("dinv", [P, G], f32, kind="ExternalInput")
    dinv2_d = nc.dram_tensor("dinv2", [P, G], f32, kind="ExternalInput")
    sqd_d = nc.dram_tensor("sqd", [P, G], f32, kind="ExternalInput")
    idx_d = nc.dram_tensor("idx", [P, sum_s], i32, kind="ExternalInput")
    outl_d = nc.dram_tensor("outl", [PC, C], f32, kind="ExternalOutput")

    gdt = bf16 if F_GBF16 else f32
    gbt = bf16 if (F_GCAST and not F_GBF16) else gdt   # gbuf dtype
    own_d = nc.dram_tensor("own", [PC, C], gdt)
    ha_d = nc.dram_tensor("ha", [NPAD, C], gdt, addr_space="Shared")
    hb_d = nc.dram_tensor("hb", [NPAD, C], gdt, addr_space="Shared")

    groups = [list(range(NCORES))]
    max_nsl = max(nb * sbar for (_, nb, sbar, _) in chunks)
    NQ = G // QUAD        # 24 full quads
    rem = G - NQ * QUAD   # 2 leftover groups
    mlp_batches = [(q * QUAD, QUAD) for q in range(NQ)]
    if rem:
        mlp_batches.append((NQ * QUAD, rem))

    with tile.TileContext(nc) as tc:
        with (
            tc.tile_pool(name="const", bufs=1) as cpool,
            tc.tile_pool(name="xin", bufs=3) as xpool,
            tc.tile_pool(name="mlp", bufs=3) as mpool,
            tc.tile_pool(name="gat", bufs=2) as gpool,
            tc.tile_pool(name="small", bufs=4) as spool,
            tc.tile_pool(name="ps", bufs=2, space="PSUM") as ppool,
            tc.tile_pool(name="ps2", bufs=2, space="PSUM") as ppool2,
        ):
            # ---- constants / persistent state ----
            w1t_sb = cpool.tile([P, 4 * H], bf16)      # [128, (kc, 256)]
            nc.sync.dma_start(
                w1t_sb[:].rearrange("p (kc h) -> p kc h", kc=4),
                w1t_d[:].rearrange("(kc p) h -> p kc h", p=P))
            w2t_sb = cpool.tile([P, 2 * C], bf16)      # [128, (jc, 64)]
            nc.sync.dma_start(
                w2t_sb[:].rearrange("p (jc c) -> p jc c", jc=2),
                w2t_d[:].rearrange("(jc p) c -> p jc c", p=P))
            b1_sb = cpool.tile([P, 2], f32)
            nc.sync.dma_start(b1_sb[:], b1_d[:].rearrange("(jc p) -> p jc", p=P))
            b2_sb = cpool.tile([P, 1], f32)
            nc.sync.dma_start(b2_sb[:C, :], b2_d[:].rearrange("(c one) -> c one", one=1))
            dinv_sb = cpool.tile([P, G], f32)
            nc.sync.dma_start(dinv_sb[:], dinv_d[:])
            dinv2_sb = cpool.tile([P, G], f32)
            nc.sync.dma_start(dinv2_sb[:], dinv2_d[:])
            sqd_sb = cpool.tile([P, G], f32)
            nc.sync.dma_start(sqd_sb[:], sqd_d[:])
            idx_sb = cpool.tile([P, sum_s], i32)
            nc.sync.dma_start(idx_sb[:], idx_d[:])
            ident = cpool.tile([P, P], bf16)
            make_identity(nc, ident[:])
            if not F_TBF:
                identf = cpool.tile([P, P], f32)
                make_identity(nc, identf[:])
            hidden = cpool.tile([P, G * C], f32)
            own_sb = cpool.tile([P, G * C], gdt)

            # ---- phase A: MLP + g0 ----
            for (gs, nb) in mlp_batches:
                W = nb * P                             # node columns
                xt_sb = xpool.tile([P, 4, QUAD * P], bf16, tag="xt")
                nc.sync.dma_start(
                    xt_sb[:, :, :W],
                    xt_d[:, gs * P:gs * P + W].rearrange(
                        "(kc p) n -> p kc n", p=P))
                h1_sb = mpool.tile([P, 2, QUAD * P], bf16, tag="h1")
                for jc in range(2):
                    ps1 = ppool.tile([P, QUAD * P], f32, tag="ps1")
                    for kc in range(4):
                        nc.tensor.matmul(
                            ps1[:, :W],
                            lhsT=w1t_sb[:, kc * H + jc * P: kc * H + (jc + 1) * P],
                            rhs=xt_sb[:, kc, :W],
                            start=(kc == 0), stop=(kc == 3))
                    nc.scalar.activation(
                        h1_sb[:, jc, :W], ps1[:, :W],
                        AF.Relu, bias=b1_sb[:, jc:jc + 1])
                ps2 = ppool.tile([P, QUAD * P], f32, tag="ps2")
                for jc in range(2):
                    nc.tensor.matmul(
                        ps2[:C, :W],
                        lhsT=w2t_sb[:, jc * C:(jc + 1) * C],
                        rhs=h1_sb[:, jc, :W],
                        start=(jc == 0), stop=(jc == 1))
                h2_sb = mpool.tile([P, QUAD * P], bf16, tag="h2")
                nc.scalar.activation(h2_sb[:C, :W], ps2[:C, :W],
                                     AF.Identity, bias=b2_sb[:C, :])
                for i in range(nb):
                    g = gs + i
                    pst = ppool2.tile([P, C], bf16 if F_TBF else f32, tag="pst")
                    if F_TBF:
                        nc.tensor.transpose(
                            pst[:], h2_sb[:C, i * P:(i + 1) * P], ident[:C, :C])
                    else:
                        h2f = mpool.tile([P, P], f32, tag="h2f")
                        nc.vector.tensor_copy(
                            h2f[:C, :P], h2_sb[:C, i * P:(i + 1) * P])
                        nc.tensor.transpose(
                            pst[:], h2f[:C, :P], identf[:C, :C])
                    nc.vector.tensor_scalar_mul(
                        own_sb[:, g * C:(g + 1) * C], pst[:],
                        dinv_sb[:, g:g + 1])
                    nc.vector.tensor_scalar_mul(
                        hidden[:, g * C:(g + 1) * C],
                        own_sb[:, g * C:(g + 1) * C], float(temps[0]))

            nc.sync.dma_start(
                own_d[:].rearrange("(g p) c -> p g c", p=P),
                own_sb[:].rearrange("p (g c) -> p g c", g=G))
            nc.gpsimd.collective_compute(
                "AllGather", mybir.AluOpType.bypass, replica_groups=groups,
                ins=[own_d[:]], outs=[ha_d[:]])

            # ---- phase B: K hops ----
            hcur, hnxt = ha_d, hb_d
            for k in range(K):
                tk = float(temps[(k + 1) % len(temps)])
                for (gs, nb, sbar, col0) in chunks:
                    nsl = nb * sbar
                    gbuf = gpool.tile([P, max_nsl * C], gbt, tag="gbuf")
                    nc.gpsimd.indirect_dma_start(
                        out=gbuf[:, :nsl * C],
                        out_offset=None,
                        in_=hcur[:],
                        in_offset=bass.IndirectOffsetOnAxis(
                            ap=idx_sb[:, col0:col0 + nsl], axis=0))
                    # log-tree fold within each group, batched across nb
                    if F_FOLD3D:
                        gv = gbuf[:, :nsl * C].rearrange(
                            "p (nb x) -> p nb x", nb=nb)
                        s = sbar
                        while s > 1:
                            h_ = s // 2
                            nc.vector.tensor_add(
                                gv[:, :, : h_ * C],
                                gv[:, :, : h_ * C],
                                gv[:, :, (s - h_) * C: s * C])
                            s -= h_
                    else:
                        for i in range(nb):
                            b0 = i * sbar * C
                            s = sbar
                            while s > 1:
                                h_ = s // 2
                                nc.vector.tensor_add(
                                    gbuf[:, b0:b0 + h_ * C],
                                    gbuf[:, b0:b0 + h_ * C],
                                    gbuf[:, b0 + (s - h_) * C:b0 + s * C])
                                s -= h_
                    if F_NOSELF:
                        # self loop: fold += g_own (still previous hop's own)
                        gfv = gbuf[:, :nsl * C].rearrange(
                            "p (nb x) -> p nb x", nb=nb)
                        nc.vector.tensor_add(
                            gfv[:, :, :C],
                            gfv[:, :, :C],
                            own_sb[:, gs * C:(gs + nb) * C].rearrange(
                                "p (nb x) -> p nb x", nb=nb))
                    for i in range(nb):
                        g = gs + i
                        nc.vector.tensor_scalar_mul(
                            own_sb[:, g * C:(g + 1) * C],
                            gbuf[:, i * sbar * C: i * sbar * C + C],
                            dinv2_sb[:, g:g + 1])
                    hv = hidden[:, gs * C:(gs + nb) * C]
                    if F_STT:
                        nc.vector.scalar_tensor_tensor(
                            out=hv, in0=own_sb[:, gs * C:(gs + nb) * C],
                            scalar=tk, in1=hv,
                            op0=ALU.mult, op1=ALU.add)
                    else:
                        gf = spool.tile([P, 32 * C], f32, tag="gf")
                        nc.vector.tensor_copy(
                            gf[:, :nb * C], own_sb[:, gs * C:(gs + nb) * C])
                        nc.vector.scalar_tensor_tensor(
                            out=hv, in0=gf[:, :nb * C],
                            scalar=tk, in1=hv,
                            op0=ALU.mult, op1=ALU.add)
                if k < K - 1:
                    nc.sync.dma_start(
                        own_d[:].rearrange("(g p) c -> p g c", p=P),
                        own_sb[:].rearrange("p (g c) -> p g c", g=G))
                    nc.gpsimd.collective_compute(
                        "AllGather", mybir.AluOpType.bypass,
                        replica_groups=groups,
                        ins=[own_d[:]], outs=[hnxt[:]])
                    hcur, hnxt = hnxt, hcur

            # ---- phase C: hidden * sqrt(deg), log_softmax, store ----
            if F_PHC:
                hs = cpool.tile([P, G * C], f32)
                for g in range(G):
                    nc.vector.tensor_scalar_mul(
                        hs[:, g * C:(g + 1) * C], hidden[:, g * C:(g + 1) * C],
                        sqd_sb[:, g:g + 1])
                nm = spool.tile([P, G], f32, tag="nm")
                nc.vector.tensor_reduce(
                    out=nm[:], in_=hs[:].rearrange("p (g c) -> p g c", g=G),
                    op=ALU.max, axis=mybir.AxisListType.X, negate=True)
                for g in range(G):
                    nc.vector.tensor_scalar_add(
                        hs[:, g * C:(g + 1) * C], hs[:, g * C:(g + 1) * C],
                        nm[:, g:g + 1])
                ex = cpool.tile([P, G * C], f32)
                nc.scalar.activation(ex[:], hs[:], AF.Exp)
                ssum = spool.tile([P, G], f32, tag="ssum")
                nc.vector.tensor_reduce(
                    out=ssum[:], in_=ex[:].rearrange("p (g c) -> p g c", g=G),
                    op=ALU.add, axis=mybir.AxisListType.X)
                lse = spool.tile([P, G], f32, tag="lse")
                nc.scalar.activation(lse[:], ssum[:], AF.Ln)
                for g in range(G):
                    nc.vector.tensor_scalar_sub(
                        hs[:, g * C:(g + 1) * C], hs[:, g * C:(g + 1) * C],
                        lse[:, g:g + 1])
                nc.sync.dma_start(
                    outl_d[:].rearrange("(g p) c -> p g c", p=P),
                    hs[:].rearrange("p (g c) -> p g c", g=G))
            else:
                for g in range(G):
                    hid = spool.tile([P, C], f32, tag="hid")
                    nc.vector.tensor_scalar_mul(
                        hid[:], hidden[:, g * C:(g + 1) * C],
                        sqd_sb[:, g:g + 1])
                    nm1 = spool.tile([P, 1], f32, tag="nm1")
                    nc.vector.reduce_max(nm1[:], hid[:],
                                         axis=mybir.AxisListType.X, negate=True)
                    ex1 = spool.tile([P, C], f32, tag="ex1")
                    nc.scalar.activation(ex1[:], hid[:], AF.Exp,
                                         bias=nm1[:, 0:1])
                    ss1 = spool.tile([P, 1], f32, tag="ss1")
                    nc.vector.reduce_sum(ss1[:], ex1[:],
                                         axis=mybir.AxisListType.X)
                    ls1 = spool.tile([P, 1], f32, tag="ls1")
                    nc.scalar.activation(ls1[:], ss1[:], AF.Ln)
                    c1 = spool.tile([P, 1], f32, tag="c1")
                    nc.vector.tensor_tensor(
                        out=c1[:], in0=nm1[:], in1=ls1[:],
                        op=mybir.AluOpType.subtract)
                    o_sb = spool.tile([P, C], f32, tag="o")
                    nc.vector.tensor_scalar_add(o_sb[:], hid[:], c1[:, 0:1])
                    nc.sync.dma_start(outl_d[g * P:(g + 1) * P, :], o_sb[:])

    nc.finalize()
    return nc


def _make_program_and_inputs(inputs, prep):
    """Build the bass program + per-core input maps + output assembler."""
    from ml_dtypes import bfloat16

    w1 = np.asarray(inputs["w1"], dtype=np.float32)
    b1 = np.asarray(inputs["b1"], dtype=np.float32)
    w2 = np.asarray(inputs["w2"], dtype=np.float32)
    b2 = np.asarray(inputs["b2"], dtype=np.float32)
    temp = np.asarray(inputs["temp"], dtype=np.float32)

    (new_id, chunks, sum_s, idx_blobs, xts,
     dinv_cols, dinv2_cols, sqd_cols) = prep

    nc = _build_program(chunks, sum_s, [float(t) for t in temp])

    w1t = np.ascontiguousarray(w1.T.astype(bfloat16))          # [512, 256]
    w2t = np.ascontiguousarray(w2.T.astype(bfloat16))          # [256, 64]
    in_maps = []
    for c in range(NCORES):
        in_maps.append({
            "xt": xts[c],
            "w1t": w1t, "b1": b1, "w2t": w2t, "b2": b2,
            "dinv": dinv_cols[c], "dinv2": dinv2_cols[c], "sqd": sqd_cols[c],
            "idx": idx_blobs[c],
        })

    def assemble(results):
        full = np.concatenate(
            [results[c]["outl"] for c in range(NCORES)], axis=0)
        return np.ascontiguousarray(full[new_id])

    return nc, in_maps, assemble


def kernel(x, w1, b1, w2, b2, temp, edge_index):
    from concourse.bass_utils import run_bass_kernel_spmd

    x = np.asarray(x, dtype=np.float32)
    inputs = {"x": x, "w1": w1, "b1": b1, "w2": w2, "b2": b2, "temp": temp,
              "edge_index": edge_index}
    prep = _host_prep(x, edge_index)
    nc, in_maps, assemble = _make_program_and_inputs(inputs, prep)

    trace = os.environ.get("KERNEL_TRACE", "0") == "1"
    res = run_bass_kernel_spmd(nc, in_maps, list(range(NCORES)), trace=trace)
    if trace:
        _profile_info["exec_time_ns"] = res.exec_time_ns
        _profile_info["mean_exec_time_ns"] = res.mean_exec_time_ns
        _profile_info["profile_json"] = res.profile_json

    return assemble([res.results[c] for c in range(NCORES)])
